# revision 5
# baseline (speedup 1.0000x reference)
"""Trainium2 Bass kernel for nn_CCMetrics (connected-component soft-Dice).

Math
----
Reference per sample: probs = softmax(y_pred, ch axis 1) with C=2 channels,
one-hot labels y in {0,1}.  Per-voxel channel sums collapse:
  psum_v = tsum_v = 1          (softmax / one-hot sum to 1 over channels)
  inter_v = probs[true_ch] = sigmoid((2y-1) * (z1 - z0))
So per segment id k (voronoi component, 0..64):
  inter_k = sum of sigmoid values over voxels with id k
  cnt_k   = voxel count with id k
  dice_k  = (2*inter_k + eps) / (2*cnt_k + eps)
  score   = mean over present k in 1..64;  output = mean over batch.

Device algorithm (per core, data-parallel over 4M voxels / 8 cores)
------------------------------------------------------------------
Pack x = g + v (integer id g plus fractional sigmoid value v in (0,1)).
Two cumulative families over the packed stream recover all bins:
  T_k = #{x >= k}                     -> cnt_k  = T_k - T_{k+1}
  R_k = sum relu(x - k)               -> with M_k = R_k - R_{k+1}:
                                         inter_k = M_k - T_{k+1}
T_k runs on the Vector engine (tensor_scalar is_ge + add-reduce, one
instruction per k); R_k runs on the Scalar engine (Relu activation with
per-partition bias -k and accumulate, one instruction per k).  The host
sums the per-core/per-partition partials and applies the O(65) dice
formula (this is the gather/unshard step; the streaming work is on-device).
"""

import os
import sys

import numpy as np

for _p in ("/opt/trn_rl_repo",):
    if os.path.isdir(_p) and _p not in sys.path:
        sys.path.insert(0, _p)

from concourse import bacc, bass, mybir, tile  # noqa: E402
from concourse import bass_utils  # noqa: E402

NUM_COMP = 64
EPS = 1e-5
B, C, H, W, D = 2, 2, 128, 128, 128
N = H * W * D                 # voxels per sample
NCORES = 8
CORES_PER_SAMPLE = NCORES // B
CHUNK = N // CORES_PER_SAMPLE  # voxels per core
P = 128
F = CHUNK // P                 # free-dim elements per partition
KMAX = NUM_COMP                # bins 1..64

# How many of the 64 value (relu) passes run on ACT; the rest run on DVE
# as two-instruction clamp+reduce pairs.  Tuned from profiles.
N_VALUES_ON_ACT = int(os.environ.get("CC_NV_ACT", "64"))
TRACE = False

_prog_cache = {}


def _build_program():
    nc = bacc.Bacc(
        "TRN2",
        target_bir_lowering=False,
        debug=False,
        enable_asserts=False,
        num_devices=NCORES,
    )
    f32 = mybir.dt.float32
    u8 = mybir.dt.uint8

    z0_d = nc.dram_tensor("z0", [P, F], f32, kind="ExternalInput").ap()
    z1_d = nc.dram_tensor("z1", [P, F], f32, kind="ExternalInput").ap()
    y_d = nc.dram_tensor("yb", [P, F], u8, kind="ExternalInput").ap()
    g_d = nc.dram_tensor("vor", [P, F], u8, kind="ExternalInput").ap()
    negk_d = nc.dram_tensor("negk", [P, KMAX + 1], f32, kind="ExternalInput").ap()
    out_d = nc.dram_tensor("out", [P, 2 * KMAX], f32, kind="ExternalOutput").ap()

    Alu = mybir.AluOpType
    Act = mybir.ActivationFunctionType

    with tile.TileContext(nc) as tc:
        with tc.tile_pool(name="main", bufs=1) as pool:
            z0 = pool.tile([P, F], f32)
            z1 = pool.tile([P, F], f32)
            yt = pool.tile([P, F], u8)
            gt = pool.tile([P, F], u8)
            negk = pool.tile([P, KMAX + 1], f32)
            nc.sync.dma_start(out=z0[:], in_=z0_d[:])
            nc.sync.dma_start(out=z1[:], in_=z1_d[:])
            nc.sync.dma_start(out=yt[:], in_=y_d[:])
            nc.sync.dma_start(out=gt[:], in_=g_d[:])
            nc.sync.dma_start(out=negk[:], in_=negk_d[:])

            s = pool.tile([P, F], f32)
            nc.vector.tensor_sub(s[:], z1[:], z0[:])
            yf = pool.tile([P, F], f32)
            nc.vector.tensor_scalar(
                out=yf[:], in0=yt[:], scalar1=2.0, scalar2=-1.0,
                op0=Alu.mult, op1=Alu.add,
            )
            t = pool.tile([P, F], f32)
            nc.vector.tensor_mul(t[:], s[:], yf[:])
            v = pool.tile([P, F], f32)
            nc.scalar.activation(
                out=v[:], in_=t[:], func=Act.Sigmoid,
                bias=negk[:, KMAX:KMAX + 1], scale=1.0,  # last column is 0.0
            )
            gf = pool.tile([P, F], f32)
            nc.vector.tensor_copy(gf[:], gt[:])
            x = pool.tile([P, F], f32)
            nc.vector.tensor_add(x[:], v[:], gf[:])

            racc = pool.tile([P, KMAX], f32)
            tacc = pool.tile([P, KMAX], f32)
            trash_a = pool.tile([P, F], f32)
            trash_d = pool.tile([P, F], f32)

            for k in range(1, KMAX + 1):
                j = k - 1
                # counts on DVE
                nc.vector.tensor_scalar(
                    out=trash_d[:], in0=x[:], scalar1=float(k), scalar2=None,
                    op0=Alu.is_ge, op1=Alu.add, accum_out=tacc[:, j:j + 1],
                )
                if j < N_VALUES_ON_ACT:
                    # values on ACT:  R_k = sum relu(x - k)
                    nc.scalar.activation(
                        out=trash_a[:], in_=x[:], func=Act.Relu,
                        bias=negk[:, j:j + 1], scale=1.0,
                        accum_out=racc[:, j:j + 1],
                    )
                else:
                    # values on DVE: clamp to [k, k+1] then sum
                    # A_k = sum min(max(x,k),k+1) = F*k + (R_k - R_{k+1})
                    nc.vector.tensor_scalar(
                        out=trash_d[:], in0=x[:], scalar1=float(k),
                        scalar2=float(k + 1), op0=Alu.max, op1=Alu.min,
                    )
                    nc.vector.tensor_scalar(
                        out=trash_a[:], in0=trash_d[:], scalar1=0.0,
                        scalar2=None, op0=Alu.add, op1=Alu.add,
                        accum_out=racc[:, j:j + 1],
                    )

            nc.sync.dma_start(out=out_d[:, 0:KMAX], in_=racc[:])
            nc.sync.dma_start(out=out_d[:, KMAX:2 * KMAX], in_=tacc[:])

    nc.compile()
    return nc


def _get_program():
    key = ("prog", N_VALUES_ON_ACT)
    if key not in _prog_cache:
        _prog_cache[key] = _build_program()
    return _prog_cache[key]


def _negk_const():
    # column j holds -(j+1) for j < KMAX (ACT bias for the relu family);
    # final column is 0.0 (bias for the sigmoid preprocessing pass)
    col = np.concatenate(
        [-np.arange(1, KMAX + 1, dtype=np.float32), np.zeros(1, np.float32)])
    return np.broadcast_to(col, (P, KMAX + 1)).copy()


def kernel(y_pred: np.ndarray, y: np.ndarray, voronoi: np.ndarray) -> np.ndarray:
    y_pred = np.asarray(y_pred, dtype=np.float32)
    y = np.asarray(y)
    voronoi = np.asarray(voronoi)

    nc = _get_program()
    negk = _negk_const()

    in_maps = []
    for c in range(NCORES):
        b = c // CORES_PER_SAMPLE
        q = c % CORES_PER_SAMPLE
        sl = slice(q * CHUNK, (q + 1) * CHUNK)
        zp = y_pred[b].reshape(C, N)
        in_maps.append({
            "z0": np.ascontiguousarray(zp[0, sl]).reshape(P, F),
            "z1": np.ascontiguousarray(zp[1, sl]).reshape(P, F),
            "yb": np.ascontiguousarray(
                y[b, 0].reshape(N)[sl]).astype(np.uint8).reshape(P, F),
            "vor": np.ascontiguousarray(
                voronoi[b].reshape(N)[sl]).astype(np.uint8).reshape(P, F),
            "negk": negk,
        })

    res = bass_utils.run_bass_kernel_spmd(
        nc, in_maps, core_ids=list(range(NCORES)), trace=TRACE,
    )
    kernel.last_results = res

    # ---- host-side gather/unshard: combine per-core partials ----
    R = np.zeros((B, KMAX + 2), dtype=np.float64)  # index 1..64; 65 stays 0
    T = np.zeros((B, KMAX + 2), dtype=np.float64)
    for c in range(NCORES):
        b = c // CORES_PER_SAMPLE
        out = np.asarray(res.results[c]["out"], dtype=np.float64)
        R[b, 1:KMAX + 1] += out[:, 0:KMAX].sum(axis=0)
        T[b, 1:KMAX + 1] += out[:, KMAX:2 * KMAX].sum(axis=0)

    scores = []
    for b in range(B):
        k = np.arange(1, KMAX + 1)
        M = R[b, k] - R[b, k + 1]
        cnt = T[b, k] - T[b, k + 1]
        inter = M - T[b, k + 1]
        dice = (2.0 * inter + EPS) / (2.0 * cnt + EPS)
        present = cnt > 0
        n_present = max(present.sum(), 1)
        scores.append(np.where(present, dice, 0.0).sum() / n_present)

    return np.float32(np.mean(scores))
